# revision 7
# baseline (speedup 1.0000x reference)
"""Trainium2 Bass kernel for nn_Attention (dense transformer block without
head split: qkv proj -> full-width attention over S=2048 -> out proj).

Sharding: 8 cores = 4 batches x 2 query-halves. Each core gets its batch's
full x (token-rotated so its own 1024 queries are rows 0..1023), computes
k/v for all 2048 tokens (replicated within the pair; no collectives), and
attention + output projection for its 1024 queries.

Layout strategy (per core):
  xT   [d, t]   f32r+bf16   via PE transposes of DMA'd x tiles
  kT   [f, t]   f32r        lhsT-chunks for QK
  qT   [f, s]   f32r        rhs for QK (s free, 512-wide)
  v    [t, f]   bf16        lhsT-chunks for PV
  dotsT [t, s]  psum        QK accumulated over f; exp -> PT bf16 (no max
                            subtraction: logits bounded well below f32 range)
  sums via ones-matmul over partition dim; 1/sum applied at final evict as a
  per-partition scalar (scatter [1,512]->[128,4] via 4 tiny K=1 matmuls).
  outT [f, s]   bf16        PV output, feeds final proj directly as lhsT
  final [s, d]  psum        + bias (broadcast-DMA) fused in evict.
"""

import numpy as np

import concourse.mybir as mybir
import concourse.tile as tile
from concourse import bacc
from concourse.bass_utils import run_bass_kernel_spmd
from concourse.masks import make_identity

f32 = mybir.dt.float32
f32r = mybir.dt.float32r
bf16 = mybir.dt.bfloat16
AF = mybir.ActivationFunctionType

P = 128
B, S, D = 4, 2048, 1024
INNER = 1024
SQ = S // 2  # queries per core
SCALE = (INNER // 16) ** -0.5  # dim_head=64 -> 0.125

DC = D // P  # 8 d-chunks
FT = INNER // P  # 8 f-tiles
TT = S // P  # 16 kv token tiles
TB = 4  # token blocks of 512 in phase A
SB = SQ // 512  # 2 query s-blocks per core
N_CORES = 8


def build_nc():
    nc = bacc.Bacc(None, target_bir_lowering=False)
    x = nc.dram_tensor("x", [S, D], f32, kind="ExternalInput")
    w_qkv = nc.dram_tensor("w_qkv", [D, 3 * INNER], f32, kind="ExternalInput")
    w_out = nc.dram_tensor("w_out", [INNER, D], f32, kind="ExternalInput")
    b_out = nc.dram_tensor("b_out", [1, D], f32, kind="ExternalInput")
    out = nc.dram_tensor("out", [SQ, D], f32, kind="ExternalOutput")

    x_t = x.rearrange("(tt p) d -> p tt d", p=P)  # [128, 16, 1024] (partition=token)
    wq_t = w_qkv.rearrange("(dc p) f -> p dc f", p=P)  # [128, 8, 3072] (partition=d)
    wo_t = w_out.rearrange("(fc p) d -> p fc d", p=P)  # [128, 8, 1024] (partition=f)

    with tile.TileContext(nc, pool_alloc_mode="queue") as tc:
        with (
            tc.tile_pool(name="persist", bufs=1) as persist,
            tc.tile_pool(name="consts", bufs=1) as consts,
        ):
            kT = persist.tile([P, FT, S], f32r)  # 64K/part
            qT = persist.tile([P, FT, SQ], f32r)  # 32K/part
            v = persist.tile([P, TT, INNER], bf16)  # 32K/part

            ident = consts.tile([P, P], f32)
            make_identity(nc, ident)
            ones_bf = consts.tile([P, 1], bf16)
            nc.vector.memset(ones_bf, 1.0)
            ones_f1 = consts.tile([1, 1], f32)
            nc.vector.memset(ones_f1, 1.0)
            ones_row = consts.tile([1, P], f32)
            nc.vector.memset(ones_row, 1.0)
            b_row = consts.tile([1, D], f32)
            nc.sync.dma_start(out=b_row, in_=b_out[:, :])
            bias_bc = consts.tile([P, D], f32)

            # ---------------- Phase A: projections ----------------
            with (
                tc.tile_pool(name="pa_sbuf", bufs=1) as pa,
                tc.tile_pool(name="pa_psum", bufs=1, space="PSUM") as pap,
            ):
                with nc.named_scope("proj"):
                    # warm-up transpose to absorb identity dep on PE
                    dummy_ps = pap.tile([P, P], f32, tag="tp", bufs=2)
                    nc.tensor.transpose(dummy_ps, ident, ident)

                    for tb in range(TB):
                        # -- transpose x block -> xTr (f32r) and xTbf (bf16)
                        xTr = pa.tile([P, DC, 512], f32r, tag="xTr", bufs=1)
                        xTbf = pa.tile([P, DC, 512], bf16, tag="xTbf", bufs=1)
                        for ti in range(4):
                            tt = tb * 4 + ti
                            x_tile = pa.tile([P, D], f32, tag="x_dma", bufs=1)
                            nc.sync.dma_start(out=x_tile, in_=x_t[:, tt])
                            for j in range(DC):
                                tp_ps = pap.tile([P, P], f32, tag="tp", bufs=2)
                                nc.tensor.transpose(
                                    tp_ps, x_tile[:, j * P : (j + 1) * P], ident
                                )
                                nc.vector.tensor_copy(
                                    xTr[:, j, ti * P : (ti + 1) * P], tp_ps
                                )
                                nc.vector.tensor_copy(
                                    xTbf[:, j, ti * P : (ti + 1) * P], tp_ps
                                )

                        # -- k and q projections (f32r)
                        for which, col0 in (("q", 0), ("k", INNER)):
                            if which == "q" and tb >= 2:
                                continue  # queries are rows 0..1023 only
                            for ft in range(FT):
                                w_stage = pa.tile(
                                    [P, DC, P], f32, tag="w_stage", bufs=2
                                )
                                nc.sync.dma_start(
                                    out=w_stage,
                                    in_=wq_t[
                                        :, :, col0 + ft * P : col0 + (ft + 1) * P
                                    ],
                                )
                                w_r = pa.tile([P, DC, P], f32r, tag="w_r", bufs=2)
                                nc.vector.tensor_copy(w_r, w_stage)
                                ps = pap.tile([P, 512], f32, tag="kq", bufs=2)
                                for dc in range(DC):
                                    nc.tensor.matmul(
                                        ps,
                                        w_r[:, dc, :],
                                        xTr[:, dc, :],
                                        start=(dc == 0),
                                        stop=(dc == DC - 1),
                                    )
                                dst = kT if which == "k" else qT
                                nc.vector.tensor_copy(
                                    dst[:, ft, tb * 512 : (tb + 1) * 512], ps
                                )

                        # -- v projection (bf16)
                        for fc in range(2):
                            wv_stage = pa.tile(
                                [P, DC, 512], f32, tag="wv_stage", bufs=1
                            )
                            nc.sync.dma_start(
                                out=wv_stage,
                                in_=wq_t[
                                    :,
                                    :,
                                    2 * INNER + fc * 512 : 2 * INNER + (fc + 1) * 512,
                                ],
                            )
                            wv_bf = pa.tile([P, DC, 512], bf16, tag="wv_bf", bufs=1)
                            nc.vector.tensor_copy(wv_bf, wv_stage)
                            for ti in range(4):
                                tt = tb * 4 + ti
                                ps = pap.tile([P, 512], f32, tag="vp", bufs=2)
                                for dc in range(DC):
                                    nc.tensor.matmul(
                                        ps,
                                        xTbf[:, dc, ti * P : (ti + 1) * P],
                                        wv_bf[:, dc, :],
                                        start=(dc == 0),
                                        stop=(dc == DC - 1),
                                    )
                                nc.vector.tensor_copy(
                                    v[:, tt, fc * 512 : (fc + 1) * 512], ps
                                )

            # ---------------- Phase B: attention + out proj ----------------
            with (
                tc.tile_pool(name="pb_sbuf", bufs=1) as pb,
                tc.tile_pool(name="pb_psum", bufs=1, space="PSUM") as pbp,
            ):
                # broadcast bias across partitions: ones[1,128].T @ b_row
                for dc2 in range(2):
                    bb_ps = pbp.tile([P, 512], f32, tag="fin", bufs=2)
                    nc.tensor.matmul(
                        bb_ps, ones_row, b_row[:, dc2 * 512 : (dc2 + 1) * 512],
                        start=True, stop=True,
                    )
                    nc.vector.tensor_copy(bias_bc[:, dc2 * 512 : (dc2 + 1) * 512], bb_ps)

                # W_out rounded to bf16, resident
                wo_bf = pb.tile([P, FT, D], bf16, tag="wo_bf", bufs=1)
                for fc in range(FT):
                    wo_stage = pb.tile([P, D], f32, tag="wo_stage", bufs=2)
                    nc.sync.dma_start(out=wo_stage, in_=wo_t[:, fc])
                    nc.vector.tensor_copy(wo_bf[:, fc], wo_stage)

                for sb in range(SB):
                    with nc.named_scope(f"qk_{sb}"):
                        PT = pb.tile([P, TT, 512], bf16, tag="PT", bufs=1)
                        for tt in range(TT):
                            dots = pbp.tile([P, 512], f32, tag="dots", bufs=2)
                            for ft in range(FT):
                                nc.tensor.matmul(
                                    dots,
                                    kT[:, ft, tt * P : (tt + 1) * P],
                                    qT[:, ft, sb * 512 : (sb + 1) * 512],
                                    start=(ft == 0),
                                    stop=(ft == FT - 1),
                                )
                            nc.scalar.activation(
                                PT[:, tt, :], dots, AF.Exp, scale=SCALE
                            )

                    with nc.named_scope(f"sum_{sb}"):
                        sum_ps = pbp.tile([1, 512], f32, tag="small", bufs=2)
                        for tt in range(TT):
                            nc.tensor.matmul(
                                sum_ps,
                                ones_bf,
                                PT[:, tt, :],
                                start=(tt == 0),
                                stop=(tt == TT - 1),
                            )
                        rcp = pb.tile([1, 512], f32, tag="rcp", bufs=2)
                        nc.vector.reciprocal(rcp, sum_ps)
                        rcp_sp = pb.tile([P, 4], f32, tag="rcp_sp", bufs=2)
                        for j in range(4):
                            scat_ps = pbp.tile([P, 1], f32, tag="small", bufs=2)
                            nc.tensor.matmul(
                                scat_ps,
                                rcp[0:1, j * P : (j + 1) * P],
                                ones_f1,
                                start=True,
                                stop=True,
                            )
                            nc.vector.tensor_copy(rcp_sp[:, j : j + 1], scat_ps)

                    with nc.named_scope(f"pv_{sb}"):
                        outT = pb.tile([P, FT, 512], bf16, tag="outT", bufs=2)
                        for ft in range(FT):
                            pv_ps = pbp.tile([P, 512], f32, tag="pv", bufs=2)
                            for tt in range(TT):
                                nc.tensor.matmul(
                                    pv_ps,
                                    v[:, tt, ft * P : (ft + 1) * P],
                                    PT[:, tt, :],
                                    start=(tt == 0),
                                    stop=(tt == TT - 1),
                                )
                            nc.vector.tensor_copy(outT[:, ft], pv_ps)

                    with nc.named_scope(f"fin_{sb}"):
                        for ss in range(4):
                            for dc2 in range(2):
                                fin_ps = pbp.tile([P, 512], f32, tag="fin", bufs=2)
                                for ft in range(FT):
                                    nc.tensor.matmul(
                                        fin_ps,
                                        outT[:, ft, ss * P : (ss + 1) * P],
                                        wo_bf[:, ft, dc2 * 512 : (dc2 + 1) * 512],
                                        start=(ft == 0),
                                        stop=(ft == FT - 1),
                                    )
                                fin_sb = pb.tile([P, 512], f32, tag="fin_sb", bufs=3)
                                nc.vector.scalar_tensor_tensor(
                                    out=fin_sb,
                                    in0=fin_ps,
                                    scalar=rcp_sp[:, ss : ss + 1],
                                    in1=bias_bc[:, dc2 * 512 : (dc2 + 1) * 512],
                                    op0=mybir.AluOpType.mult,
                                    op1=mybir.AluOpType.add,
                                )
                                r0 = sb * 512 + ss * P
                                nc.sync.dma_start(
                                    out=out[r0 : r0 + P, dc2 * 512 : (dc2 + 1) * 512],
                                    in_=fin_sb,
                                )

    nc.compile()
    return nc


_NC_CACHE = {}


def _get_nc():
    if "nc" not in _NC_CACHE:
        _NC_CACHE["nc"] = build_nc()
    return _NC_CACHE["nc"]


def kernel(x, W_qkv, W_out, b_out):
    x = np.ascontiguousarray(np.asarray(x, dtype=np.float32))
    W_qkv = np.ascontiguousarray(np.asarray(W_qkv, dtype=np.float32))
    W_out = np.ascontiguousarray(np.asarray(W_out, dtype=np.float32))
    b_out = np.ascontiguousarray(np.asarray(b_out, dtype=np.float32)).reshape(1, D)

    nc = _get_nc()
    in_maps = []
    for c in range(N_CORES):
        b, h = divmod(c, 2)
        xb = x[b]
        x_c = np.concatenate([xb[SQ * h :], xb[: SQ * h]], axis=0) if h else xb
        in_maps.append(
            {"x": np.ascontiguousarray(x_c), "w_qkv": W_qkv, "w_out": W_out,
             "b_out": b_out}
        )

    res = run_bass_kernel_spmd(nc, in_maps, core_ids=list(range(N_CORES)))
    full = np.empty((B, S, D), dtype=np.float32)
    for c in range(N_CORES):
        b, h = divmod(c, 2)
        full[b, SQ * h : SQ * (h + 1)] = res.results[c]["out"]
    return full


# revision 9
# speedup vs baseline: 1.2481x; 1.2481x over previous
"""Trainium2 Bass kernel for nn_Attention (dense transformer block without
head split: qkv proj -> full-width attention over S=2048 -> out proj).

Sharding: 8 cores = 4 batches x 2 query-halves. Each core gets its batch's
full x (token-rotated so its own 1024 queries are rows 0..1023), computes
k/v for all 2048 tokens (replicated within the pair; no collectives), and
attention + output projection for its 1024 queries.

Precision: q/k projection and QK^T in f32r (TF32), v/PV/out-proj in bf16.
Weights are DMA'd directly as f32r / host-pre-cast bf16 (no staging copies).

Layout (per core):
  xT    [d, t]  f32r+bf16  via PE transposes of DMA'd x tiles
  kT    [f, t]  f32r       lhsT-chunks for QK
  qT    [f, s]  f32r       rhs for QK (s free, 512-wide)
  v     [t, f]  bf16       lhsT-chunks for PV
  dotsT [t, s]  psum       QK accumulated over f; ACT exp -> PT bf16 (no max
                           subtraction: logits bounded far below f32 range)
  softmax sums via ones-matmul over the partition dim; 1/sum applied at the
  final evict as a per-partition scalar (scatter [1,512] -> [128,4] via 4
  tiny K=1 matmuls). outT [f, s] bf16 feeds the out-proj directly as lhsT;
  bias is broadcast with a K=1 ones-matmul and fused into the final evict.
"""

import numpy as np

import concourse.mybir as mybir
import concourse.tile as tile
from concourse import bacc
from concourse.bass_utils import run_bass_kernel_spmd

f32 = mybir.dt.float32
f32r = mybir.dt.float32r
bf16 = mybir.dt.bfloat16
AF = mybir.ActivationFunctionType

P = 128
B, S, D = 4, 2048, 1024
INNER = 1024
SQ = S // 2  # queries per core
SCALE = (INNER // 16) ** -0.5  # dim_head=64 -> 0.125

DC = D // P  # 8 d-chunks
FT = INNER // P  # 8 f-tiles
TT = S // P  # 16 kv token tiles
TB = 4  # token blocks of 512 in phase A
SB = SQ // 512  # 2 query s-blocks per core
N_CORES = 8


def build_nc():
    nc = bacc.Bacc(None, target_bir_lowering=False)
    x = nc.dram_tensor("x", [S, D], f32r, kind="ExternalInput")
    w_qk = nc.dram_tensor("w_qk", [D, 2 * INNER], f32r, kind="ExternalInput")
    w_v = nc.dram_tensor("w_v", [D, INNER], bf16, kind="ExternalInput")
    w_o = nc.dram_tensor("w_o", [INNER, D], bf16, kind="ExternalInput")
    b_out = nc.dram_tensor("b_out", [1, D], f32, kind="ExternalInput")
    ident_in = nc.dram_tensor("ident", [P, P], f32r, kind="ExternalInput")
    out = nc.dram_tensor("out", [SQ, D], f32, kind="ExternalOutput")

    x_t = x.rearrange("(tt p) d -> p tt d", p=P)  # [128, 16, 1024] (part=token)
    wqk_t = w_qk.rearrange("(dc p) f -> p dc f", p=P)  # [128, 8, 2048] (part=d)
    wv_t = w_v.rearrange("(dc p) f -> p dc f", p=P)  # [128, 8, 1024]
    wo_t = w_o.rearrange("(fc p) d -> p fc d", p=P)  # [128, 8, 1024] (part=f)

    with tile.TileContext(nc, pool_alloc_mode="queue") as tc:
        with (
            tc.tile_pool(name="persist", bufs=1) as persist,
            tc.tile_pool(name="consts", bufs=1) as consts,
        ):
            kT = persist.tile([P, FT, S], f32r)  # 64K/part
            qT = persist.tile([P, FT, SQ], f32r)  # 32K/part
            v = persist.tile([P, TT, INNER], bf16)  # 32K/part

            ident = consts.tile([P, P], f32r)
            nc.sync.dma_start(out=ident, in_=ident_in[:, :])
            ones_bf = consts.tile([P, 1], bf16)
            nc.vector.memset(ones_bf, 1.0)
            ones_f1 = consts.tile([1, 1], f32)
            nc.vector.memset(ones_f1, 1.0)
            ones_row = consts.tile([1, P], f32)
            nc.vector.memset(ones_row, 1.0)
            b_row = consts.tile([1, D], f32)
            nc.sync.dma_start(out=b_row, in_=b_out[:, :])
            bias_bc = consts.tile([P, D], f32)

            # ---------------- Phase A: projections ----------------
            with (
                tc.tile_pool(name="pa_sbuf", bufs=1) as pa,
                tc.tile_pool(name="pa_psum", bufs=1, space="PSUM") as pap,
            ):
                with nc.named_scope("proj"):
                    # warm-up transpose absorbs the identity dep on PE
                    dummy_ps = pap.tile([P, P], f32r, tag="tp", bufs=2)
                    nc.tensor.transpose(dummy_ps, ident, ident)

                    for tb in range(TB):
                        # -- transpose x block -> xTr (f32r) and xTbf (bf16)
                        xTr = pa.tile([P, DC, 512], f32r, tag="xTr", bufs=1)
                        xTbf = pa.tile([P, DC, 512], bf16, tag="xTbf", bufs=1)
                        for ti in range(4):
                            tt = tb * 4 + ti
                            x_tile = pa.tile([P, D], f32r, tag="x_dma", bufs=2)
                            nc.sync.dma_start(out=x_tile, in_=x_t[:, tt])
                            for j in range(DC):
                                tp_ps = pap.tile([P, P], f32r, tag="tp", bufs=2)
                                nc.tensor.transpose(
                                    tp_ps, x_tile[:, j * P : (j + 1) * P], ident
                                )
                                nc.vector.tensor_copy(
                                    xTr[:, j, ti * P : (ti + 1) * P], tp_ps
                                )
                                nc.scalar.activation(
                                    xTbf[:, j, ti * P : (ti + 1) * P], tp_ps, AF.Copy
                                )

                        # -- k and q projections (f32r)
                        for which, col0 in (("q", 0), ("k", INNER)):
                            if which == "q" and tb >= 2:
                                continue  # queries are rows 0..1023 only
                            for ft in range(FT):
                                w_r = pa.tile([P, DC, P], f32r, tag="w_r", bufs=4)
                                nc.sync.dma_start(
                                    out=w_r,
                                    in_=wqk_t[
                                        :, :, col0 + ft * P : col0 + (ft + 1) * P
                                    ],
                                )
                                ps = pap.tile([P, 512], f32, tag="kq", bufs=3)
                                for dc in range(DC):
                                    nc.tensor.matmul(
                                        ps,
                                        w_r[:, dc, :],
                                        xTr[:, dc, :],
                                        start=(dc == 0),
                                        stop=(dc == DC - 1),
                                    )
                                dst = kT if which == "k" else qT
                                nc.vector.tensor_copy(
                                    dst[:, ft, tb * 512 : (tb + 1) * 512], ps
                                )

                        # -- v projection (bf16)
                        for fc in range(2):
                            wv_bf = pa.tile([P, DC, 512], bf16, tag="wv_bf", bufs=2)
                            nc.sync.dma_start(
                                out=wv_bf,
                                in_=wv_t[:, :, fc * 512 : (fc + 1) * 512],
                            )
                            for ti in range(4):
                                tt = tb * 4 + ti
                                ps = pap.tile([P, 512], f32, tag="vp", bufs=3)
                                for dc in range(DC):
                                    nc.tensor.matmul(
                                        ps,
                                        xTbf[:, dc, ti * P : (ti + 1) * P],
                                        wv_bf[:, dc, :],
                                        start=(dc == 0),
                                        stop=(dc == DC - 1),
                                    )
                                nc.vector.tensor_copy(
                                    v[:, tt, fc * 512 : (fc + 1) * 512], ps
                                )

            # ---------------- Phase B: attention + out proj ----------------
            with (
                tc.tile_pool(name="pb_sbuf", bufs=1) as pb,
                tc.tile_pool(name="pb_psum", bufs=1, space="PSUM") as pbp,
            ):
                # broadcast bias across partitions: ones[1,128].T @ b_row
                for dc2 in range(2):
                    bb_ps = pbp.tile([P, 512], f32, tag="fin", bufs=2)
                    nc.tensor.matmul(
                        bb_ps, ones_row, b_row[:, dc2 * 512 : (dc2 + 1) * 512],
                        start=True, stop=True,
                    )
                    nc.vector.tensor_copy(
                        bias_bc[:, dc2 * 512 : (dc2 + 1) * 512], bb_ps
                    )

                wo_bf = pb.tile([P, FT, D], bf16, tag="wo_bf", bufs=1)
                nc.sync.dma_start(out=wo_bf, in_=wo_t)

                for sb in range(SB):
                    with nc.named_scope(f"qk_{sb}"):
                        PT = pb.tile([P, TT, 512], bf16, tag="PT", bufs=2)
                        for tt in range(TT):
                            dots = pbp.tile([P, 512], f32, tag="dots", bufs=2)
                            for ft in range(FT):
                                nc.tensor.matmul(
                                    dots,
                                    kT[:, ft, tt * P : (tt + 1) * P],
                                    qT[:, ft, sb * 512 : (sb + 1) * 512],
                                    start=(ft == 0),
                                    stop=(ft == FT - 1),
                                )
                            nc.scalar.activation(
                                PT[:, tt, :], dots, AF.Exp, scale=SCALE
                            )

                    with nc.named_scope(f"sum_{sb}"):
                        sum_ps = pbp.tile([1, 512], f32, tag="small", bufs=2)
                        for tt in range(TT):
                            nc.tensor.matmul(
                                sum_ps,
                                ones_bf,
                                PT[:, tt, :],
                                start=(tt == 0),
                                stop=(tt == TT - 1),
                            )
                        rcp = pb.tile([1, 512], f32, tag="rcp", bufs=1)
                        nc.vector.reciprocal(rcp, sum_ps)
                        rcp_sp = pb.tile([P, 4], f32, tag="rcp_sp", bufs=2)
                        for j in range(4):
                            scat_ps = pbp.tile([P, 1], f32, tag="small", bufs=2)
                            nc.tensor.matmul(
                                scat_ps,
                                rcp[0:1, j * P : (j + 1) * P],
                                ones_f1,
                                start=True,
                                stop=True,
                            )
                            nc.vector.tensor_copy(rcp_sp[:, j : j + 1], scat_ps)

                    with nc.named_scope(f"pv_{sb}"):
                        outT = pb.tile([P, FT, 512], bf16, tag="outT", bufs=2)
                        for ft in range(FT):
                            pv_ps = pbp.tile([P, 512], f32, tag="pv", bufs=2)
                            for tt in range(TT):
                                nc.tensor.matmul(
                                    pv_ps,
                                    v[:, tt, ft * P : (ft + 1) * P],
                                    PT[:, tt, :],
                                    start=(tt == 0),
                                    stop=(tt == TT - 1),
                                )
                            nc.vector.tensor_copy(outT[:, ft], pv_ps)

                    with nc.named_scope(f"fin_{sb}"):
                        for ss in range(4):
                            for dc2 in range(2):
                                fin_ps = pbp.tile([P, 512], f32, tag="fin", bufs=2)
                                for ft in range(FT):
                                    nc.tensor.matmul(
                                        fin_ps,
                                        outT[:, ft, ss * P : (ss + 1) * P],
                                        wo_bf[:, ft, dc2 * 512 : (dc2 + 1) * 512],
                                        start=(ft == 0),
                                        stop=(ft == FT - 1),
                                    )
                                fin_sb = pb.tile([P, 512], f32, tag="fin_sb", bufs=2)
                                nc.vector.scalar_tensor_tensor(
                                    out=fin_sb,
                                    in0=fin_ps,
                                    scalar=rcp_sp[:, ss : ss + 1],
                                    in1=bias_bc[:, dc2 * 512 : (dc2 + 1) * 512],
                                    op0=mybir.AluOpType.mult,
                                    op1=mybir.AluOpType.add,
                                )
                                r0 = sb * 512 + ss * P
                                nc.sync.dma_start(
                                    out=out[r0 : r0 + P, dc2 * 512 : (dc2 + 1) * 512],
                                    in_=fin_sb,
                                )

    nc.compile()
    return nc


_NC_CACHE = {}


def _get_nc():
    if "nc" not in _NC_CACHE:
        _NC_CACHE["nc"] = build_nc()
    return _NC_CACHE["nc"]


def _prep_weights(W_qkv, W_out, b_out):
    import ml_dtypes

    W_qkv = np.asarray(W_qkv, dtype=np.float32)
    w_qk = np.ascontiguousarray(W_qkv[:, : 2 * INNER])
    w_v = np.ascontiguousarray(W_qkv[:, 2 * INNER :].astype(ml_dtypes.bfloat16))
    w_o = np.ascontiguousarray(
        np.asarray(W_out, dtype=np.float32).astype(ml_dtypes.bfloat16)
    )
    b = np.ascontiguousarray(np.asarray(b_out, dtype=np.float32)).reshape(1, D)
    ident = np.eye(P, dtype=np.float32)
    return w_qk, w_v, w_o, b, ident


def make_in_maps(x, W_qkv, W_out, b_out):
    x = np.asarray(x, dtype=np.float32)
    w_qk, w_v, w_o, b, ident = _prep_weights(W_qkv, W_out, b_out)
    in_maps = []
    for c in range(N_CORES):
        bi, h = divmod(c, 2)
        xb = x[bi]
        x_c = np.concatenate([xb[SQ * h :], xb[: SQ * h]], axis=0) if h else xb
        in_maps.append(
            {
                "x": np.ascontiguousarray(x_c),
                "w_qk": w_qk,
                "w_v": w_v,
                "w_o": w_o,
                "b_out": b,
                "ident": ident,
            }
        )
    return in_maps


def kernel(x, W_qkv, W_out, b_out):
    nc = _get_nc()
    in_maps = make_in_maps(x, W_qkv, W_out, b_out)
    res = run_bass_kernel_spmd(nc, in_maps, core_ids=list(range(N_CORES)))
    full = np.empty((B, S, D), dtype=np.float32)
    for c in range(N_CORES):
        bi, h = divmod(c, 2)
        full[bi, SQ * h : SQ * (h + 1)] = res.results[c]["out"]
    return full
